# revision 1
# baseline (speedup 1.0000x reference)
"""Trainium2 Bass kernel for nn_BondLenConstrain.

Contract: kernel(**inputs) takes the FULL (unsharded) inputs of
reference.setup_inputs() and returns the full [64, 4, 2048, 2] float32
resiEnergy tensor.  Data-parallel over the batch axis across 8 NeuronCores
(8 batches per core).

Host (numpy, indexing only): scatter atoms into dense residue grids exactly
like the reference, build the `todo` mask, gather the tiny per-residue-type
tables into per-residue coefficient planes (masked pairs get all-zero
coefficients -> device formula returns exactly 0), transpose coords to a
plane-contiguous blocked layout, and broadcast the (identical) nalt lanes
of the output on assembly.

Device math per residue pair (r-1, r):
    v2 = CA_r - N_r, v1 = C_{r-1} - N_r, v3 = CA_{r-1} - C_{r-1}
    bond  f1 = sqrt(d11) = exp(0.5 ln d11)
    angle theta = pi/2 - sign(c) * arctan(|c|/s),  s = sqrt(dxx*d11 - c^2)
          arctan over [0,inf) via  t' = arctan(exp(-|ln(|c|/s)|)) in [0,pi/4]
          (ScalarE arctan domain is [-pi/2, pi/2])
    score_d = min(((f_d - mu_d) / (sqrt2 sigma_d))^2, ln(q_d/(EPS sqrt(pi))))
    e = s_w * sum_d score_d
Signs are folded into squared terms; normalisations go through exp/ln
(ScalarE Rsqrt/Reciprocal are disallowed).

Scheduling notes:
  * plane-contiguous free-dim layouts -> every DVE op streams unit-stride.
  * two chunks pipeline DMA/DVE/ACT/GPSIMD; per-batch DMAs spread queues.
  * walrus reloads the ACT function table on every Ln/Exp/Arctan function
    CHANGE (Square/Sign are fillers present in every set).  Forcing a
    globally grouped Ln/Exp/Arctan order minimizes loads but serializes the
    scoring tail and measured slower; the free-running per-chunk order wins.
    A dummy Ln hides the first table load inside the DMA fill.
"""

import os
import numpy as np

PAD = -999.0
PAD_I = -999
NB, MC, MR = 64, 4, 2048
NALT = 2
NCORES = 8
BPC = NB // NCORES            # batches per core
CH = int(os.environ.get("BLC_CHUNKS", "2"))  # pipeline chunks per core
KC = 4 * CH                   # blocks per (batch, chain) across full chain
R = MR // KC                  # residues (pairs) per partition
EPS = 1e-12
CL = 1.0 / (EPS * np.sqrt(np.pi))

_PROGRAM_CACHE = {}
LAST_RESULT = None            # BassKernelResults of the last run (for test.py)
TRACE = bool(int(os.environ.get("BLC_TRACE", "0")))


def _build_program():
    import concourse.bass as bass
    import concourse.tile as tile
    from concourse import bacc, mybir
    from concourse.bass import _add_dep_helper

    dt = mybir.dt.float32
    Alu = mybir.AluOpType
    Act = mybir.ActivationFunctionType

    nc = bacc.Bacc("TRN2", target_bir_lowering=False, debug=False)
    # const AP for the Sign bias (maps c == 0 to sign +1)
    _sgn_eps = 1e-35
    _ct = nc.alloc_sbuf_tensor("const-f32-sgneps", [128, 1], dt)
    nc.gpsimd.memset(_ct.ap(), _sgn_eps)
    nc.const_aps.aps[(dt, _sgn_eps)] = _ct.ap()
    _pi4 = float(np.pi / 4)
    _cq = nc.alloc_sbuf_tensor("const-f32-pi4", [128, 1], dt)
    nc.gpsimd.memset(_cq.ap(), _pi4)
    nc.const_aps.aps[(dt, _pi4)] = _cq.ap()
    nc.all_engine_barrier()

    G_t = nc.declare_dram_parameter("g", [BPC, MC, KC, 9, R + 1], dt,
                                    isOutput=False)
    P_t = nc.declare_dram_parameter("pr", [BPC, MC, KC, 9, R], dt,
                                    isOutput=False)
    O_t = nc.declare_dram_parameter("out", [BPC, MC, MR], dt, isOutput=True)

    bc = BPC // CH            # batches per chunk
    bufs = min(CH, 2)
    S = R + 1                 # slots per coord plane

    with tile.TileContext(nc) as tc:
        with (
            tc.tile_pool(name="px", bufs=bufs) as px,
            tc.tile_pool(name="pp", bufs=bufs) as pp,
            tc.tile_pool(name="ps", bufs=bufs) as ps,
        ):
            st1, st2, st3 = [], [], []
            loads = []
            # one DMA per tensor per chunk, all on the sync HWDGE ring:
            # extra dma_starts cost ~1.2us sequencer issue each and measured
            # slower in every split variant (2-way, 4-way, cross-ring)
            for c in range(CH):
                b0 = c * bc
                X = px.tile([128, 9 * S], dt, tag="x")
                P = pp.tile([128, 9 * R], dt, tag="p")
                nc.sync.dma_start(X[:], G_t[b0:b0 + bc])
                nc.sync.dma_start(P[:], P_t[b0:b0 + bc])
                loads.append((X, P))

            # dummy Ln after the DMA issues: its ACT table load fills the
            # DMA wait instead of delaying transfers or real ACT work
            dscr = ps.tile([128, 1], dt, tag="dummy")
            nc.scalar.activation(dscr[:], _ct.ap(), Act.Ln)

            # -------- phase 1: geometry, up to the Ln input ----------------
            for c in range(CH):
                b0 = c * bc
                X, P = loads[c]
                # difference vectors, plane-contiguous [v2|v1|v3] x (x,y,z)
                V = px.tile([128, 9 * R], dt, tag="v")
                Vv = V[:].rearrange("p (v c l) -> p v c l", v=3, c=3)
                Xv = X[:].rearrange("p (n l) -> p n l", n=9)
                nc.vector.tensor_sub(Vv[:, 0], Xv[:, 3:6, 1:S], Xv[:, 0:3, 1:S])
                nc.vector.tensor_sub(Vv[:, 1], Xv[:, 6:9, 0:R], Xv[:, 0:3, 1:S])
                nc.vector.tensor_sub(Vv[:, 2], Xv[:, 3:6, 0:R], Xv[:, 6:9, 0:R])

                SQ = px.tile([128, 9 * R], dt, tag="sq")
                nc.scalar.activation(SQ[:], V[:], Act.Square)
                SQv = SQ[:].rearrange("p (v c l) -> p v c l", v=3, c=3)
                D1 = ps.tile([128, 3 * R], dt, tag="d1")   # [d22|d11|d33]
                D1v = D1[:].rearrange("p (v l) -> p v l", v=3)
                nc.vector.tensor_add(D1v, SQv[:, :, 0], SQv[:, :, 1])
                nc.vector.tensor_add(D1v, D1v, SQv[:, :, 2])
                CP = ps.tile([128, 6 * R], dt, tag="cp")   # [v1*v2|v3*v1]
                nc.vector.tensor_mul(CP[:], V[:, 3 * R:9 * R], V[:, 0:6 * R])
                CPv = CP[:].rearrange("p (m c l) -> p m c l", m=2, c=3)
                DC = ps.tile([128, 2 * R], dt, tag="dc")   # [d12|d31]
                DCv = DC[:].rearrange("p (m l) -> p m l", m=2)
                nc.vector.tensor_add(DCv, CPv[:, :, 0], CPv[:, :, 1])
                nc.vector.tensor_add(DCv, DCv, CPv[:, :, 2])

                M = ps.tile([128, 2 * R], dt, tag="m")
                nc.vector.tensor_mul(M[:, 0:R], D1[:, 0:R], D1[:, R:2 * R])
                nc.vector.tensor_mul(M[:, R:2 * R], D1[:, 2 * R:3 * R],
                                     D1[:, R:2 * R])
                PSQ = ps.tile([128, 2 * R], dt, tag="psq")
                nc.scalar.activation(PSQ[:], DC[:], Act.Square)
                SG = ps.tile([128, 2 * R], dt, tag="sg")
                nc.scalar.activation(SG[:], DC[:], Act.Sign, bias=1e-35)
                S2 = ps.tile([128, 2 * R], dt, tag="s2")
                nc.vector.tensor_sub(S2[:], M[:], PSQ[:])
                LNIN = ps.tile([128, 5 * R], dt, tag="lnin")
                nc.vector.tensor_scalar_max(LNIN[:, 0:2 * R], S2[:], 1e-30)
                nc.vector.tensor_mul(LNIN[:, 2 * R:4 * R], DC[:], SG[:])
                nc.vector.tensor_scalar_max(
                    LNIN[:, 2 * R:4 * R], LNIN[:, 2 * R:4 * R], 1e-35)
                nc.vector.tensor_scalar_max(LNIN[:, 4 * R:5 * R],
                                            D1[:, R:2 * R], 1e-30)
                LNO = ps.tile([128, 5 * R], dt, tag="lno")
                ln_i = nc.scalar.activation(LNO[:], LNIN[:], Act.Ln)
                RT = ps.tile([128, 2 * R], dt, tag="rt")
                nc.vector.scalar_tensor_tensor(
                    RT[:], LNO[:, 0:2 * R], -0.5, LNO[:, 2 * R:4 * R],
                    op0=Alu.mult, op1=Alu.add)
                SR = ps.tile([128, 2 * R], dt, tag="sr")
                nc.scalar.activation(SR[:], RT[:], Act.Sign)
                ABSR = ps.tile([128, 2 * R], dt, tag="absr")
                nc.vector.tensor_mul(ABSR[:], RT[:], SR[:])
                EN = ps.tile([128, 2 * R], dt, tag="en")
                en_i = nc.scalar.activation(EN[:], ABSR[:], Act.Exp, scale=-1.0)
                F1 = ps.tile([128, R], dt, tag="f1")
                f1_i = nc.scalar.activation(F1[:], LNO[:, 4 * R:5 * R],
                                            Act.Exp, scale=0.5)
                TP = ps.tile([128, 2 * R], dt, tag="tp")
                tp_i = nc.scalar.activation(TP[:], EN[:], Act.Arctan)
                TB = ps.tile([128, 2 * R], dt, tag="tb")
                nc.scalar.activation(TB[:], TP[:], Act.Identity,
                                     bias=float(np.pi / 4), scale=-1.0)
                TC = ps.tile([128, 2 * R], dt, tag="tc")
                nc.vector.tensor_mul(TC[:], SR[:], TB[:])
                AV = ps.tile([128, 2 * R], dt, tag="av")
                nc.vector.tensor_mul(AV[:], SG[:], P[:, R:3 * R])
                W = ps.tile([128, 3 * R], dt, tag="w")
                nc.vector.tensor_mul(W[:, 0:R], F1[:], P[:, 3 * R:4 * R])
                nc.vector.scalar_tensor_tensor(
                    W[:, R:3 * R], TC[:], np.pi / 4, P[:, 4 * R:6 * R],
                    op0=Alu.add, op1=Alu.mult)
                U = ps.tile([128, 3 * R], dt, tag="u")
                nc.vector.tensor_sub(U[:, 0:R], W[:, 0:R], P[:, 0:R])
                nc.vector.tensor_sub(U[:, R:3 * R], W[:, R:3 * R], AV[:])
                Z = ps.tile([128, 3 * R], dt, tag="z")
                nc.scalar.activation(Z[:], U[:], Act.Square)
                ZC = ps.tile([128, 3 * R], dt, tag="zc")
                nc.vector.tensor_tensor(ZC[:], Z[:], P[:, 6 * R:9 * R],
                                        op=Alu.min)
                E = ps.tile([128, R], dt, tag="e")
                nc.gpsimd.tensor_add(E[:], ZC[:, 0:R], ZC[:, R:2 * R])
                nc.gpsimd.tensor_add(E[:], E[:], ZC[:, 2 * R:3 * R])
                nc.sync.dma_start(
                    O_t[b0:b0 + bc].rearrange("b c (k l) -> b c k l", k=KC),
                    E[:])

    return nc


def _get_program():
    if "nc" not in _PROGRAM_CACHE:
        nc = _build_program()
        nc.finalize()   # Bacc: register allocation / DCE / wait legalization
        _PROGRAM_CACHE["nc"] = nc
    return _PROGRAM_CACHE["nc"]


def _host_prep(atom_description, coords, mean, std, weight):
    ad = np.asarray(atom_description)
    coords = np.asarray(coords, dtype=np.float32)
    b, ch, rs, rn, an = (ad[:, i] for i in range(5))
    valid = (b >= 0) & (b < NB) & (ch >= 0) & (ch < MC) & (rs >= 0) & (rs < MR)

    def scat3(mask):
        A = np.full((NB, MC, MR, 3), PAD, np.float32)
        m = mask & valid
        A[b[m], ch[m], rs[m]] = coords[m]
        return A

    Narr, CAarr, Carr = scat3(an == 0), scat3(an == 1), scat3(an == 2)
    seq = np.full((NB, MC, MR), PAD_I, np.int64)
    m = (an == 1) & valid
    seq[b[m], ch[m], rs[m]] = rn[m]

    todo = ((Narr[:, :, 1:, 0] != PAD) & (Carr[:, :, :-1, 0] != PAD)
            & (CAarr[:, :, 1:, 0] != PAD) & (CAarr[:, :, :-1, 0] != PAD)
            & (seq[:, :, 1:] != PAD_I) & (seq[:, :, :-1] != PAD_I))
    sidx = np.clip(np.where(todo, seq[:, :, 1:], 0), 0, 19)

    w0 = float(np.asarray(weight).reshape(-1)[0])
    s_w = 1.0 - np.tanh(-w0)
    sq = np.sqrt(s_w)
    mu = np.asarray(mean, np.float64)
    sd = np.asarray(std, np.float64)
    q = 1.0 / (sd * np.sqrt(2.0))
    tab = np.empty((20, 9))
    tab[:, 0] = mu[:, 0] * q[:, 0] * sq
    tab[:, 1] = (np.pi / 2 - mu[:, 1]) * q[:, 1] * sq
    tab[:, 2] = (mu[:, 2] - np.pi / 2) * q[:, 2] * sq
    tab[:, 3:6] = q * sq
    tab[:, 6:9] = s_w * np.maximum(np.log(CL * q), 0.0)
    tab = tab.astype(np.float32)

    params = np.zeros((NB, MC, MR, 9), np.float32)
    params[:, :, 1:, :] = tab[sidx] * todo[..., None].astype(np.float32)
    # blocked coefficient-plane layout [NB, MC, KC, 9, R]
    pblk = np.ascontiguousarray(
        params.reshape(NB, MC, KC, R, 9).transpose(0, 1, 2, 4, 3))

    G = np.zeros((NB, MC, MR + 1, 9), np.float32)
    G[:, :, 1:, 0:3] = Narr
    G[:, :, 1:, 3:6] = CAarr
    G[:, :, 1:, 6:9] = Carr
    # blocked plane-contiguous with halo: GB[b,c,k,p,l] = G[b,c,k*R+l,p]
    GB = np.empty((NB, MC, KC, 9, R + 1), np.float32)
    for k in range(KC):
        GB[:, :, k] = G[:, :, k * R:k * R + R + 1, :].transpose(0, 1, 3, 2)
    return GB, pblk


def _install_ntff_hook():
    """The agent image's antenv lacks axon_hooks; synthesize it so
    trace=True can reach the terminal's NRT profiler (dev-only path)."""
    import sys, types
    if "antenv.axon_hooks" in sys.modules:
        return True
    try:
        import antenv
        mod = types.ModuleType("antenv.axon_hooks")
        mod._hook = None

        def set_axon_ntff_profile_hook(h):
            mod._hook = h

        def get_axon_ntff_profile_hook():
            return mod._hook

        mod.set_axon_ntff_profile_hook = set_axon_ntff_profile_hook
        mod.get_axon_ntff_profile_hook = get_axon_ntff_profile_hook
        sys.modules["antenv.axon_hooks"] = mod
        antenv.axon_hooks = mod
        from trn_agent_boot.trn_boot import _ntff_profile_via_ctypes
        mod._hook = _ntff_profile_via_ctypes("/opt/axon/libaxon_pjrt.so")
        return True
    except Exception as e:  # pragma: no cover - profiling is best-effort
        print(f"ntff hook install failed: {e}")
        return False


def kernel(**inputs):
    global LAST_RESULT
    from concourse.bass_utils import run_bass_kernel_spmd
    if TRACE:
        _install_ntff_hook()

    G, pblk = _host_prep(
        inputs["atom_description"], inputs["coords"],
        inputs["mean"], inputs["std"], inputs["weight"])

    nc = _get_program()
    in_maps = [
        {"g": np.ascontiguousarray(G[i * BPC:(i + 1) * BPC]),
         "pr": np.ascontiguousarray(pblk[i * BPC:(i + 1) * BPC])}
        for i in range(NCORES)
    ]
    res = run_bass_kernel_spmd(nc, in_maps, list(range(NCORES)), trace=TRACE)
    LAST_RESULT = res
    e = np.concatenate([res.results[i]["out"] for i in range(NCORES)], axis=0)
    e = e.reshape(NB, MC, MR)
    out = np.repeat(e[..., None], NALT, axis=-1)
    return np.ascontiguousarray(out.astype(np.float32))



# revision 5
# speedup vs baseline: 1.2103x; 1.2103x over previous
"""Trainium2 Bass kernel for nn_BondLenConstrain (v2, fp16 pipeline).

Contract: kernel(**inputs) takes the FULL (unsharded) inputs of
reference.setup_inputs() and returns the full [64, 4, 2048, 2] float32
resiEnergy tensor.  Data-parallel over the batch axis across 8 NeuronCores
(8 batches per core).

Host (numpy, indexing only): scatter atoms into dense residue grids,
build the `todo` mask, gather the tiny per-residue-type tables into
per-pair fp16 coefficient planes (masked pairs get all-zero coefficients
-> device returns exactly 0), and lay out coords (f32, prescaled by 1/16)
in a plane-contiguous blocked layout with a one-slot halo.

Device math per residue pair (r-1, r), fp16 unless noted:
    v2 = CA_r - N_r, v1 = C_{r-1} - N_r, v3 = CA_{r-1} - C_{r-1}
    (subtractions read f32 coords, write fp16 - avoids cancellation loss)
    d11,d22,d33,c1,c2 via one self-mul + one cross-mul + two segmented adds
    ln pass (f32 out) over [d11 | s1^2 | s2^2 | |c1| | |c2|]
    f1 = exp(0.5 ln d11)
    L = ln|c| - 0.5 ln(s^2)  (= ln t, t = |c|/s)   [f32]
    phi = pi/4 + arctan(tanh(L/2))    <- Gudermannian identity replaces the
        sign/exp range-reduction dance: arctan(t) = pi/4 + arctan(tanh(ln(t)/2))
    U = [f1*QB - mu0*Q0 | phi*(+-Q) - sign(c)*(pi/2-mu)*Q]   (x-> -sx, squared)
    E = sum_d min(U^2, CAP)
ACT function tables: one manual load of {ln,exp} (natural_log_exp_and_others)
up front and one of {tanh,arctan} (sigmoid_and_others) between the chunk
fronts and tails -> 1 hidden + 1 mid-stream table load total (vs 6 before).
"""

import os
import numpy as np

PAD = -999.0
PAD_I = -999
NB, MC, MR = 64, 4, 2048
NALT = 2
NCORES = 8
BPC = NB // NCORES            # batches per core
CH = int(os.environ.get("BLC_CHUNKS", "2"))  # pipeline chunks per core
KC = 4 * CH                   # blocks per (batch, chain) across full chain
R = MR // KC                  # residues (pairs) per partition
S = R + 1                     # coord slots per plane (halo)
EPS = 1e-12
CL = 1.0 / (EPS * np.sqrt(np.pi))
SC = 1.0 / 16.0               # coord prescale (fp16 range safety)

_PROGRAM_CACHE = {}
LAST_RESULT = None            # BassKernelResults of the last run (for test.py)
TRACE = bool(int(os.environ.get("BLC_TRACE", "0")))


def _build_program():
    import concourse.bass as bass
    import concourse.tile as tile
    from concourse import bacc, mybir
    from concourse.hw_specs import get_activation_tables

    f16 = mybir.dt.float16
    f32 = mybir.dt.float32
    Alu = mybir.AluOpType
    Act = mybir.ActivationFunctionType

    nc = bacc.Bacc("TRN2", target_bir_lowering=False, debug=False)
    # const AP for the Sign bias (maps c == 0 to sign +1)
    _ct = nc.alloc_sbuf_tensor("const-f32-1em6", [128, 1], f32)
    nc.gpsimd.memset(_ct.ap(), 1e-6)
    nc.const_aps.aps[(f32, 1e-6)] = _ct.ap()
    nc.all_engine_barrier()

    X_t = nc.declare_dram_parameter("cx", [BPC, MC, KC, 9, S], f32,
                                    isOutput=False)
    P_t = nc.declare_dram_parameter("pr", [BPC, MC, KC, 9, R], f16,
                                    isOutput=False)
    O_t = nc.declare_dram_parameter("out", [BPC, MC, MR], f16, isOutput=True)

    table_names = list(get_activation_tables(nc.m.arch).keys())

    nc._manual_act_loads = set()

    def load_act_set(name):
        inst = mybir.InstLoadActFuncSet(
            name=nc.get_next_instruction_name(), ins=[], outs=[],
            act_func_set_id=table_names.index(name))
        nc._manual_act_loads.add(inst.name)
        nc.scalar.add_instruction(inst)

    bc = BPC // CH            # batches per chunk
    bufs = min(CH, 2)
    PI4 = float(np.pi / 4)

    with tile.TileContext(nc) as tc:
        with (
            tc.tile_pool(name="px", bufs=bufs) as px,
            tc.tile_pool(name="ps", bufs=bufs) as ps,
        ):
            # ---- DMA loads for all chunks up front (one sync ring) -------
            loads = []
            for c in range(CH):
                b0 = c * bc
                X = px.tile([128, 9 * S], f32, tag="x")
                P = px.tile([128, 9 * R], f16, tag="p")
                nc.sync.dma_start(X[:], X_t[b0:b0 + bc])
                nc.sync.dma_start(P[:], P_t[b0:b0 + bc])
                loads.append((X, P))

            # {ln, exp} table; load hides inside the DMA fill
            load_act_set("natural_log_exp_and_others")

            # ---- phase 1 (front): geometry up to ln/exp, per chunk -------
            st = []
            for c in range(CH):
                X, P = loads[c]
                Xv = X[:].rearrange("p (n l) -> p n l", n=9)
                V = px.tile([128, 9 * R], f16, tag="v")     # [v2 | v1 | v3]
                Vv = V[:].rearrange("p (n l) -> p n l", n=3)
                nc.vector.tensor_sub(Vv[:, 0], Xv[:, 3:6, 1:S], Xv[:, 0:3, 1:S])
                nc.vector.tensor_sub(Vv[:, 1], Xv[:, 6:9, 0:R], Xv[:, 0:3, 1:S])
                nc.vector.tensor_sub(Vv[:, 2], Xv[:, 3:6, 0:R], Xv[:, 6:9, 0:R])

                SQCP = px.tile([128, 15 * R], f16, tag="sqcp")
                # squares on ACT (table-free filler), cross-products on DVE
                nc.scalar.square(SQCP[:, 0:9 * R], V[:])
                nc.vector.tensor_mul(SQCP[:, 9 * R:15 * R],
                                     V[:, 3 * R:9 * R], V[:, 0:6 * R])
                DD = ps.tile([128, 5 * R], f16, tag="dd")  # [d22|d11|d33|c1|c2]
                DDv = DD[:].rearrange("p (g l) -> p g l", g=5)
                SQv = SQCP[:].rearrange("p (g c l) -> p g c l", g=5, c=3)
                nc.vector.tensor_add(DDv, SQv[:, :, 0], SQv[:, :, 1])
                nc.vector.tensor_add(DDv, DDv, SQv[:, :, 2])

                LNIN = ps.tile([128, 5 * R], f16, tag="lnin")
                MT = ps.tile([128, 2 * R], f16, tag="mt")   # [m1 | m2]
                nc.vector.tensor_mul(MT[:, 0:R], DD[:, 0:R], DD[:, R:2 * R])
                nc.vector.tensor_mul(MT[:, R:2 * R], DD[:, 2 * R:3 * R],
                                     DD[:, R:2 * R])
                PSQ = ps.tile([128, 2 * R], f16, tag="psq")  # [c1^2 | c2^2]
                nc.scalar.square(PSQ[:], DD[:, 3 * R:5 * R])
                SG = ps.tile([128, 2 * R], f16, tag="sg")
                nc.scalar.activation(SG[:], DD[:, 3 * R:5 * R], Act.Sign,
                                     bias=1e-6)
                nc.scalar.activation(LNIN[:, 3 * R:5 * R], DD[:, 3 * R:5 * R],
                                     Act.Abs)
                nc.vector.tensor_sub(LNIN[:, R:3 * R], MT[:], PSQ[:])
                nc.vector.tensor_scalar_max(LNIN[:, 0:R], DD[:, R:2 * R],
                                            6e-8)
                nc.vector.tensor_scalar_max(LNIN[:, R:3 * R],
                                            LNIN[:, R:3 * R], 6e-8)
                nc.vector.tensor_scalar_max(LNIN[:, 3 * R:5 * R],
                                            LNIN[:, 3 * R:5 * R], 1e-6)
                LNO = ps.tile([128, 5 * R], f32, tag="lno")
                nc.scalar.activation(LNO[:], LNIN[:], Act.Ln)
                T3 = ps.tile([128, 3 * R], f16, tag="t3")   # [f1 | phi1 | phi2]
                nc.scalar.activation(T3[:, 0:R], LNO[:, 0:R], Act.Exp,
                                     scale=0.5)
                st.append((P, SG, LNO, T3))

            # {tanh, arctan} table between fronts and tails
            load_act_set("sigmoid_and_others")

            # ---- phase 2 (tail): angle + scoring, per chunk --------------
            tails = []
            for c in range(CH):
                P, SG, LNO, T3 = st[c]
                L = ps.tile([128, 2 * R], f32, tag="l")
                nc.vector.scalar_tensor_tensor(
                    L[:], LNO[:, R:3 * R], -0.5, LNO[:, 3 * R:5 * R],
                    op0=Alu.mult, op1=Alu.add)
                AV = ps.tile([128, 2 * R], f16, tag="av")
                nc.vector.tensor_mul(AV[:], SG[:], P[:, 4 * R:6 * R])
                TH = ps.tile([128, 2 * R], f16, tag="th")
                nc.scalar.activation(TH[:], L[:], Act.Tanh, scale=0.5)
                nc.scalar.activation(T3[:, R:3 * R], TH[:], Act.Arctan)
                tails.append((L, AV, TH))

            for c in range(CH):
                b0 = c * bc
                P, SG, LNO, T3 = st[c]
                L, AV, TH = tails[c]
                nc.vector.tensor_scalar_add(T3[:, R:3 * R], T3[:, R:3 * R],
                                            PI4)
                W = ps.tile([128, 3 * R], f16, tag="w")
                nc.vector.tensor_mul(W[:], T3[:], P[:, 0:3 * R])
                U = ps.tile([128, 3 * R], f16, tag="u")
                nc.vector.tensor_sub(U[:, 0:R], W[:, 0:R], P[:, 3 * R:4 * R])
                nc.vector.tensor_sub(U[:, R:3 * R], W[:, R:3 * R], AV[:])
                Z = ps.tile([128, 3 * R], f16, tag="z")
                nc.vector.tensor_mul(Z[:], U[:], U[:])
                ZC = ps.tile([128, 3 * R], f16, tag="zc")
                nc.vector.tensor_tensor(ZC[:], Z[:], P[:, 6 * R:9 * R],
                                        op=Alu.min)
                E = ps.tile([128, R], f16, tag="e")
                eng = nc.gpsimd if c < CH - 1 else nc.vector
                eng.tensor_add(E[:], ZC[:, 0:R], ZC[:, R:2 * R])
                eng.tensor_add(E[:], E[:], ZC[:, 2 * R:3 * R])
                nc.sync.dma_start(
                    O_t[b0:b0 + bc].rearrange("b c (k l) -> b c k l", k=KC),
                    E[:])

    return nc


def _strip_auto_act_loads(nc):
    """Drop the table loads Bacc's insert_act_table_loads added: its
    first-match set choice ping-pongs between {ln}/{exp}/{arctan} sets.
    Our two manual loads (ln+exp set, tanh+arctan set) cover every
    activation in program order.  The pass runs after semaphore
    generation, so its loads carry no sync info and are safe to remove."""
    from concourse import mybir
    manual = getattr(nc, "_manual_act_loads", set())
    removed = 0
    for f in nc.m.functions:
        for blk in f.blocks:
            keep = []
            for inst in blk.instructions:
                if (isinstance(inst, mybir.InstLoadActFuncSet)
                        and inst.name not in manual):
                    si = inst.sync_info
                    if si is not None and (len(si.on_wait) or len(si.on_update)):
                        keep.append(inst)  # has sync; leave it alone
                        continue
                    removed += 1
                    continue
                keep.append(inst)
            blk.instructions[:] = keep
    return removed


def _get_program():
    if "nc" not in _PROGRAM_CACHE:
        nc = _build_program()
        nc.finalize()   # Bacc: register allocation / DCE / wait legalization
        if bool(int(os.environ.get("BLC_STRIP_LOADS", "1"))):
            _strip_auto_act_loads(nc)
        _PROGRAM_CACHE["nc"] = nc
    return _PROGRAM_CACHE["nc"]


def _host_prep(atom_description, coords, mean, std, weight):
    ad = np.asarray(atom_description)
    coords = np.asarray(coords, dtype=np.float32)
    b, ch, rs, rn, an = (ad[:, i] for i in range(5))
    valid = (b >= 0) & (b < NB) & (ch >= 0) & (ch < MC) & (rs >= 0) & (rs < MR)

    def scat3(mask):
        A = np.full((NB, MC, MR, 3), PAD, np.float32)
        m = mask & valid
        A[b[m], ch[m], rs[m]] = coords[m]
        return A

    Narr, CAarr, Carr = scat3(an == 0), scat3(an == 1), scat3(an == 2)
    seq = np.full((NB, MC, MR), PAD_I, np.int64)
    m = (an == 1) & valid
    seq[b[m], ch[m], rs[m]] = rn[m]

    todo = ((Narr[:, :, 1:, 0] != PAD) & (Carr[:, :, :-1, 0] != PAD)
            & (CAarr[:, :, 1:, 0] != PAD) & (CAarr[:, :, :-1, 0] != PAD)
            & (seq[:, :, 1:] != PAD_I) & (seq[:, :, :-1] != PAD_I))
    sidx = np.clip(np.where(todo, seq[:, :, 1:], 0), 0, 19)

    w0 = float(np.asarray(weight).reshape(-1)[0])
    s_w = 1.0 - np.tanh(-w0)
    sqw = np.sqrt(s_w)
    mu = np.asarray(mean, np.float64)
    sd = np.asarray(std, np.float64)
    qd = 1.0 / (sd * np.sqrt(2.0))
    Q = qd * sqw
    tab = np.empty((20, 9))
    tab[:, 0] = (1.0 / SC) * Q[:, 0]            # QB
    tab[:, 1] = Q[:, 1]                         # QS1
    tab[:, 2] = -Q[:, 2]                        # QS2
    tab[:, 3] = mu[:, 0] * Q[:, 0]              # MU0*Q0
    tab[:, 4] = (np.pi / 2 - mu[:, 1]) * Q[:, 1]  # MQ1
    tab[:, 5] = (np.pi / 2 - mu[:, 2]) * Q[:, 2]  # MQ2
    tab[:, 6:9] = s_w * np.maximum(np.log(CL * qd), 0.0)  # CAP
    tab = tab.astype(np.float32)

    params = np.zeros((NB, MC, MR, 9), np.float32)
    params[:, :, 1:, :] = tab[sidx] * todo[..., None].astype(np.float32)
    pblk = np.ascontiguousarray(
        params.reshape(NB, MC, KC, R, 9).transpose(0, 1, 2, 4, 3)
    ).astype(np.float16)

    G = np.zeros((NB, MC, MR + 1, 9), np.float32)
    G[:, :, 1:, 0:3] = np.where(Narr == PAD, 0.0, Narr) * SC
    G[:, :, 1:, 3:6] = np.where(CAarr == PAD, 0.0, CAarr) * SC
    G[:, :, 1:, 6:9] = np.where(Carr == PAD, 0.0, Carr) * SC
    # blocked plane-contiguous with halo: GB[b,c,k,p,l] = G[b,c,k*R+l,p]
    GB = np.empty((NB, MC, KC, 9, S), np.float32)
    for k in range(KC):
        GB[:, :, k] = G[:, :, k * R:k * R + S, :].transpose(0, 1, 3, 2)
    return GB, pblk


def _install_ntff_hook():
    """The agent image's antenv lacks axon_hooks; synthesize it so
    trace=True can reach the terminal's NRT profiler (dev-only path)."""
    import sys, types
    if "antenv.axon_hooks" in sys.modules:
        return True
    try:
        import antenv
        mod = types.ModuleType("antenv.axon_hooks")
        mod._hook = None

        def set_axon_ntff_profile_hook(h):
            mod._hook = h

        def get_axon_ntff_profile_hook():
            return mod._hook

        mod.set_axon_ntff_profile_hook = set_axon_ntff_profile_hook
        mod.get_axon_ntff_profile_hook = get_axon_ntff_profile_hook
        sys.modules["antenv.axon_hooks"] = mod
        antenv.axon_hooks = mod
        from trn_agent_boot.trn_boot import _ntff_profile_via_ctypes
        mod._hook = _ntff_profile_via_ctypes("/opt/axon/libaxon_pjrt.so")
        return True
    except Exception as e:  # pragma: no cover - profiling is best-effort
        print(f"ntff hook install failed: {e}")
        return False


def kernel(**inputs):
    global LAST_RESULT
    from concourse.bass_utils import run_bass_kernel_spmd
    if TRACE:
        _install_ntff_hook()

    G, pblk = _host_prep(
        inputs["atom_description"], inputs["coords"],
        inputs["mean"], inputs["std"], inputs["weight"])

    nc = _get_program()
    in_maps = [
        {"cx": np.ascontiguousarray(G[i * BPC:(i + 1) * BPC]),
         "pr": np.ascontiguousarray(pblk[i * BPC:(i + 1) * BPC])}
        for i in range(NCORES)
    ]
    res = run_bass_kernel_spmd(nc, in_maps, list(range(NCORES)), trace=TRACE)
    LAST_RESULT = res
    e = np.concatenate([res.results[i]["out"] for i in range(NCORES)], axis=0)
    e = e.astype(np.float32).reshape(NB, MC, MR)
    out = np.repeat(e[..., None], NALT, axis=-1)
    return np.ascontiguousarray(out.astype(np.float32))
